# revision 35
# baseline (speedup 1.0000x reference)
"""SANet-style attention (nn_Attention_1382979470038) on 8 TRN2 NeuronCores.

Sharding: 8 cores = 4 batches x 2 content-token halves (sequence parallel on
N, style tokens replicated within each pair).  No collectives: each core
computes output columns [C=512, N_loc=2048] of its batch independently.

Per-core math (M = 4096 style tokens, N_loc = 2048 content tokens):
  instance-norm folded into conv weights:  F = (f_w . rstd_c) @ x_half + f_b'
  G  = (g_w . rstd_s) @ style + g_b'      [C, M]   (c on partitions)
  Ht = style^T @ h_w^T + h_b              [M, C]   (m on partitions)
  St = G^T F                               [M, N]   (m on partitions)
  P  = exp(St)            (no max-subtraction: |S| <~ 30 is fp32-safe)
  den[n] = sum_m P[m,n]   (all-ones stationary matmul -> broadcast rows)
  out = out_w @ ((Ht^T P) . (1/den)) + out_b

float32r (FP22) matmuls for convs+scores; bf16 for the post-exp apply.
Inputs are host-rearranged to k-major [128, ...] layouts so each logical
load is a single DMA (DMA dispatch costs ~0.6us each on the sync queue).
"""

import sys

sys.path.insert(0, "/opt/trn_rl_repo")

import numpy as np

import concourse.bass as bass
import concourse.tile as tile
from concourse import mybir

P = 128
C = 512
HW = 4096
NLOC = 2048
EPS = 1e-5
KT = C // P        # 4 k-tiles of 128 channels
NCH = NLOC // 512  # 4 n-chunks of 512
MCH = HW // 512    # 8 m-chunks of 512
MT = HW // P       # 32 m-tiles of 128

F32 = mybir.dt.float32
F32R = mybir.dt.float32r
BF16 = mybir.dt.bfloat16

AF = mybir.ActivationFunctionType
ALU = mybir.AluOpType


def _r(ap):
    return ap.bitcast(F32R)


def build_nc(hoist=True):
    nc = bass.Bass()
    # k-major layouts: [128, KT * cols]; column block k holds rows k*128..
    cAp = nc.declare_dram_parameter("cA", [P, KT * NLOC], F32, isOutput=False)
    cBp = nc.declare_dram_parameter("cB", [P, KT * NLOC], F32, isOutput=False)
    stp = nc.declare_dram_parameter("style", [P, KT * HW], F32, isOutput=False)
    fwp_ = nc.declare_dram_parameter("fwT", [P, KT * C], F32, isOutput=False)
    gwp_ = nc.declare_dram_parameter("gwT", [P, KT * C], F32, isOutput=False)
    hwp_ = nc.declare_dram_parameter("hwT", [P, KT * C], F32, isOutput=False)
    owp_ = nc.declare_dram_parameter("owT", [P, KT * C], F32, isOutput=False)
    pbp = nc.declare_dram_parameter("pbias", [P, 3 * KT], F32, isOutput=False)
    rcp = nc.declare_dram_parameter("rowconst", [1, P + C], F32,
                                    isOutput=False)
    out = nc.declare_dram_parameter("out", [C, NLOC], F32, isOutput=True)

    with tile.TileContext(nc) as tc:
        _build(tc, cAp, cBp, stp, fwp_, gwp_, hwp_, owp_, pbp, rcp, out)
    if hoist:
        _hoist_excess_waits(nc)
    return nc


# Walrus caps sync-wait commands per instruction (Activation/TensorScalar fit
# only one).  Hoist excess waits onto injected same-engine NOPs placed just
# before the instruction: engines execute in order, so semantics match.
def _hoist_excess_waits(nc):
    ctr = [0]

    def mknop(engine, debug, waits, updates):
        ctr[0] += 1
        return mybir.InstNoOp(
            name=f"WH-{ctr[0]}", opcode="NoOp", engine=engine, debug=debug,
            ins=[], outs=[],
            sync_info=mybir.SyncInfo(on_wait=waits, on_update=updates),
        )

    for fn in nc.m.functions:
        for blk in fn.blocks:
            newl = []
            changed = False
            for inst in blk.instructions:
                si = getattr(inst, "sync_info", None)
                if si is not None and si.on_wait and len(si.on_wait) > 1:
                    waits = list(si.on_wait)
                    keep, hoist = waits[-1:], waits[:-1]
                    eng = getattr(inst, "engine", None)
                    for w in hoist:
                        newl.append(mknop(eng, inst.debug, [w], []))
                    inst.sync_info = mybir.SyncInfo(
                        on_wait=keep, on_update=list(si.on_update))
                    changed = True
                newl.append(inst)
            if changed:
                blk.instructions = newl


def _build(tc, cAp, cBp, stp, fwTp, gwTp, hwTp, owTp, pbp, rcp, out):
    nc = tc.nc
    from contextlib import ExitStack

    ctx = ExitStack()
    with ctx:
        # ---------- long-lived pools ----------
        small = ctx.enter_context(tc.tile_pool(name="small", bufs=96))
        cons = ctx.enter_context(tc.tile_pool(name="cons", bufs=1))
        fpool = ctx.enter_context(tc.tile_pool(name="fpool", bufs=4))
        owpool = ctx.enter_context(tc.tile_pool(name="owpool", bufs=1))
        # PSUM pools (8 banks: 4 + 2 + 1 + 1)
        stps = ctx.enter_context(tc.tile_pool(name="stps", bufs=4,
                                              space="PSUM"))
        mmps = ctx.enter_context(tc.tile_pool(name="mmps", bufs=2,
                                              space="PSUM"))
        denps = ctx.enter_context(tc.tile_pool(name="denps", bufs=1,
                                               space="PSUM"))
        biasps = ctx.enter_context(tc.tile_pool(name="biasps", bufs=1,
                                                space="PSUM"))

        # == phase 0: style streamed ONCE -> Ht conv + style stats ==========
        htpool = ctx.enter_context(tc.tile_pool(name="htpool", bufs=1))
        Ht_sb = htpool.tile([P, MT * C], BF16, tag="Ht")
        spieces = [[] for _ in range(KT)]

        def stat_piece(tslice, ncols, scratch_pool):
            """One [P, ncols] slice -> (sum, sumsq) [P,1] tiles."""
            s = small.tile([P, 1], F32, tag="stat")
            nc.vector.reduce_sum(s[:], tslice.bitcast(F32),
                                 axis=mybir.AxisListType.X)
            q = small.tile([P, 1], F32, tag="stat")
            scr = scratch_pool.tile([P, ncols], F32, tag="scr")
            nc.scalar.activation(scr[:], tslice.bitcast(F32), AF.Square,
                                 accum_out=q[:])
            return s, q

        def stat_final(pieces, total):
            ssum = pieces[0][0]
            for s, _ in pieces[1:]:
                t2 = small.tile([P, 1], F32, tag="stat")
                nc.vector.tensor_add(t2[:], ssum[:], s[:])
                ssum = t2
            qsum = pieces[0][1]
            for _, q in pieces[1:]:
                t2 = small.tile([P, 1], F32, tag="stat")
                nc.vector.tensor_add(t2[:], qsum[:], q[:])
                qsum = t2
            mean = small.tile([P, 1], F32R, tag="stat")
            nc.vector.tensor_scalar(mean[:], ssum[:], 1.0 / total, None,
                                    op0=ALU.mult)
            m2 = small.tile([P, 1], F32, tag="stat")
            nc.vector.tensor_mul(m2[:], mean[:].bitcast(F32),
                                 mean[:].bitcast(F32))
            v = small.tile([P, 1], F32, tag="stat")
            nc.vector.scalar_tensor_tensor(
                out=v[:], in0=m2[:], scalar=-float(total), in1=qsum[:],
                op0=ALU.mult, op1=ALU.add)
            varp = small.tile([P, 1], F32, tag="stat")
            nc.vector.tensor_scalar(varp[:], v[:], 1.0 / (total - 1), EPS,
                                    op0=ALU.mult, op1=ALU.add)
            std = small.tile([P, 1], F32, tag="stat")
            nc.scalar.activation(std[:], varp[:], AF.Sqrt)
            rstd = small.tile([P, 1], F32, tag="stat")
            nc.vector.reciprocal(rstd[:], std[:])
            return mean, rstd

        def bias_fixup(w_t, mean_tiles, pb_col0):
            """b'[o] = pbias[:, col] - sum_c w_scaled[c,o]*mean[c]."""
            outb = []
            for j in range(KT):
                ps = biasps.tile([P, 1], F32, tag="biasps")
                for k in range(KT):
                    nc.tensor.matmul(
                        ps[:],
                        w_t[:, k * C + j * P: k * C + (j + 1) * P]
                        .bitcast(F32),
                        mean_tiles[k][:].bitcast(F32),
                        start=(k == 0), stop=(k == KT - 1))
                bb = small.tile([P, 1], F32, tag="pb")
                nc.vector.tensor_sub(
                    bb[:], pb_sb[:, pb_col0 + j: pb_col0 + j + 1], ps[:])
                outb.append(bb)
            return outb

        def acc_piece(accs, k, piece):
            """Running (sum, sumsq) accumulation per k-tile."""
            if accs[k] is None:
                accs[k] = piece
            else:
                s0, q0 = accs[k]
                s1, q1 = piece
                s2 = small.tile([P, 1], F32, tag="stat")
                nc.vector.tensor_add(s2[:], s0[:], s1[:])
                q2 = small.tile([P, 1], F32, tag="stat")
                nc.vector.tensor_add(q2[:], q0[:], q1[:])
                accs[k] = (s2, q2)

        saccs = [None] * KT
        caccs = [None] * KT
        with tc.tile_pool(name="hwp", bufs=1) as hwp, \
             tc.tile_pool(name="sspool", bufs=2) as sspool, \
             tc.tile_pool(name="ssbfp", bufs=2) as ssbfp, \
             tc.tile_pool(name="fwpp", bufs=1) as fwpp, \
             tc.tile_pool(name="cbig", bufs=1) as cbig, \
             tc.tile_pool(name="cquart", bufs=2) as cquart, \
             tc.tile_pool(name="scratch0", bufs=2) as scratch0:
            hw_s = hwp.tile([P, KT * C], F32R, tag="hwT")
            nc.sync.dma_start(hw_s[:], hwTp[:, :].bitcast(F32R))
            hw_bf = hwp.tile([P, KT * C], BF16, tag="hwbf")
            nc.vector.tensor_copy(hw_bf[:], hw_s[:].bitcast(F32))
            rc_sb = cons.tile([1, P + C], F32R, tag="rc")
            nc.sync.dma_start(rc_sb[:], rcp[:, :].bitcast(F32R))
            ones_row = rc_sb[:, :P]
            hb_s = rc_sb[:, P:]
            pb_sb = cons.tile([P, 3 * KT], F32, tag="pb_sb")
            nc.sync.dma_start(pb_sb[:], pbp[:, :])
            ones_bf = cons.tile([P, P], BF16, tag="ones_bf")
            nc.vector.memset(ones_bf[:], 1.0)
            hb_bf = cons.tile([1, C], BF16, tag="hb_bf")
            nc.vector.tensor_copy(hb_bf[:], hb_s.bitcast(F32))

            stp3 = stp[:, :].rearrange("p (k m) -> p k m", k=KT)
            cA3 = cAp[:, :].rearrange("p (k m) -> p k m", k=KT)
            cB3 = cBp[:, :].rearrange("p (k m) -> p k m", k=KT)
            cA_s = cbig.tile([P, KT * NLOC], F32R, tag="cA")
            fw_raw = fwpp.tile([P, KT * C], F32, tag="fwr")
            fw_s = fwpp.tile([P, KT * C], F32R, tag="fws")
            cq = []
            for ch in range(MCH):
                sc = sspool.tile([P, KT * 512], F32R, tag="ss",
                                 name=f"ss{ch}")
                nc.sync.dma_start(
                    sc[:], stp3[:, :, ch * 512:(ch + 1) * 512].bitcast(F32R))
                scbf = ssbfp.tile([P, KT * 512], BF16, tag="ssbf",
                                  name=f"ssbf{ch}")
                nc.vector.tensor_copy(scbf[:], sc[:].bitcast(F32))
                if ch == 1:
                    nc.sync.dma_start(cA_s[:], cAp[:, :].bitcast(F32R))
                    nc.sync.dma_start(fw_raw[:], fwTp[:, :])
                if ch in (2, 3, 4, 5):
                    q = cquart.tile([P, KT * 512], F32, tag="cq",
                                    name=f"cq{ch}")
                    nc.sync.dma_start(
                        q[:], cB3[:, :, (ch - 2) * 512:(ch - 1) * 512])
                    cq.append(q)
                for mi in range(4):
                    mt = ch * 4 + mi
                    ps = mmps.tile([P, 512], F32, tag="mmps")
                    for k in range(KT):
                        nc.tensor.matmul(
                            ps[:],
                            scbf[:, k * 512 + mi * P: k * 512 + (mi + 1) * P],
                            hw_bf[:, k * C:(k + 1) * C],
                            start=(k == 0), stop=False)
                    nc.tensor.matmul(ps[:], ones_bf[:1, :], hb_bf[:],
                                     start=False, stop=True)
                    nc.scalar.activation(
                        Ht_sb[:, mt * C:(mt + 1) * C], ps[:], AF.Copy)
                for k in range(KT):
                    acc_piece(saccs, k, stat_piece(
                        sc[:, k * 512:(k + 1) * 512], 512, scratch0))
                if 3 <= ch <= 6:  # content pieces trail the cB quarter DMAs
                    qq = cq[ch - 3]
                    for k in range(KT):
                        acc_piece(caccs, k, stat_piece(
                            qq[:, k * 512:(k + 1) * 512], 512, scratch0))
                if ch >= 4:  # cA pieces (cA arrives by mid-phase)
                    for k in range(KT):
                        acc_piece(caccs, k, stat_piece(
                            cA_s[:, k * NLOC + (ch - 4) * 512:
                                 k * NLOC + (ch - 3) * 512], 512, scratch0))
            smean, srstd = [], []
            for k in range(KT):
                mean, rstd = stat_final([saccs[k]], HW)
                smean.append(mean)
                srstd.append(rstd)

            # ============= content finalize + F conv =============
            with tc.tile_pool(name="scratch1", bufs=2) as scratch1:
                cmean, crstd = [], []
                for k in range(KT):
                    pends = [caccs[k]]
                    # remaining cA pieces (chunks NCH-? .. already covered 4,5,
                    # 6,7 -> (ch-4) in 0..3 == all 4 chunks of 512? NLOC=2048
                    # has 4 chunks; covered. )
                    mean, rstd = stat_final(pends, HW)
                    cmean.append(mean)
                    crstd.append(rstd)
                    nc.vector.tensor_scalar_mul(
                        fw_s[:, k * C:(k + 1) * C],
                        fw_raw[:, k * C:(k + 1) * C], crstd[k][:])
                fbp = bias_fixup(fw_s, cmean, 0)
                F_sb = [fpool.tile([P, NLOC], F32R, tag="F", name=f"F{k}")
                        for k in range(KT)]
                for ch in range(NCH):
                    for j in range(KT):
                        ps = stps.tile([P, 512], F32, tag="stps")
                        for k in range(KT):
                            nc.tensor.matmul(
                                ps[:],
                                _r(fw_s[:, k * C + j * P:
                                       k * C + (j + 1) * P]),
                                _r(cA_s[:, k * NLOC + ch * 512:
                                        k * NLOC + (ch + 1) * 512]),
                                start=(k == 0), stop=(k == KT - 1))
                        nc.scalar.activation(
                            F_sb[j][:, ch * 512:(ch + 1) * 512], ps[:],
                            AF.Identity, bias=fbp[j][:])

        # ========== phase 2: G conv (style re-streamed in chunk sets) ======
        gpool = ctx.enter_context(tc.tile_pool(name="gpool", bufs=4))
        G_sb = [gpool.tile([P, HW], F32R, tag="G", name=f"G{k}")
                for k in range(KT)]
        ghw_stack = ExitStack()
        ghwp = ghw_stack.enter_context(tc.tile_pool(name="ghwp", bufs=1))
        gw_raw = ghwp.tile([P, KT * C], F32, tag="gwr")
        nc.sync.dma_start(gw_raw[:], gwTp[:, :])
        gw_s = ghwp.tile([P, KT * C], F32R, tag="gws")
        for k in range(KT):
            nc.vector.tensor_scalar_mul(
                gw_s[:, k * C:(k + 1) * C],
                gw_raw[:, k * C:(k + 1) * C], srstd[k][:])
        gbp = bias_fixup(gw_s, smean, KT)
        with tc.tile_pool(name="schunk2", bufs=3) as schunk2:
            for ch in range(MCH):
                sc = schunk2.tile([P, KT * 512], F32R, tag="schunk",
                                  name=f"sg{ch}")
                nc.sync.dma_start(
                    sc[:],
                    stp[:, :].rearrange("p (k m) -> p k m", k=KT)
                    [:, :, ch * 512:(ch + 1) * 512].bitcast(F32R))
                for j in range(KT):
                    ps = stps.tile([P, 512], F32, tag="stps")
                    for k in range(KT):
                        nc.tensor.matmul(
                            ps[:],
                            _r(gw_s[:, k * C + j * P: k * C + (j + 1) * P]),
                            _r(sc[:, k * 512:(k + 1) * 512]),
                            start=(k == 0), stop=(k == KT - 1))
                    nc.scalar.activation(
                        G_sb[j][:, ch * 512:(ch + 1) * 512], ps[:],
                        AF.Identity, bias=gbp[j][:])
        ghw_stack.close()

        # ================= phase 3: attention =================
        ow_s = owpool.tile([P, KT * C], F32R, tag="owT")
        nc.sync.dma_start(ow_s[:], owTp[:, :].bitcast(F32R))
        with tc.tile_pool(name="expp", bufs=1) as expp, \
             tc.tile_pool(name="styp", bufs=4) as styp, \
             tc.tile_pool(name="rdenp", bufs=1) as rdenp, \
             tc.tile_pool(name="outp", bufs=2) as outp:
            for ch in range(NCH):
                exp_t = expp.tile([P, MT * 512], BF16, tag="exp")
                den = denps.tile([P, 512], F32, tag="den")
                for mt in range(MT):
                    ps = stps.tile([P, 512], F32, tag="stps")
                    for k in range(KT):
                        nc.tensor.matmul(
                            ps[:], _r(G_sb[k][:, mt * P:(mt + 1) * P]),
                            _r(F_sb[k][:, ch * 512:(ch + 1) * 512]),
                            start=(k == 0), stop=(k == KT - 1))
                    nc.scalar.activation(
                        exp_t[:, mt * 512:(mt + 1) * 512], ps[:], AF.Exp)
                    nc.tensor.matmul(
                        den[:], ones_bf[:],
                        exp_t[:, mt * 512:(mt + 1) * 512],
                        start=(mt == 0), stop=(mt == MT - 1))
                rden = rdenp.tile([P, 512], F32, tag="rden")
                nc.vector.reciprocal(rden[:], den[:])
                sty = []
                for j in range(KT):
                    ps = mmps.tile([P, 512], F32, tag="mmps")
                    for mt in range(MT):
                        nc.tensor.matmul(
                            ps[:],
                            Ht_sb[:, mt * C + j * P: mt * C + (j + 1) * P],
                            exp_t[:, mt * 512:(mt + 1) * 512],
                            start=(mt == 0), stop=(mt == MT - 1))
                    s_t = styp.tile([P, 512], F32R, tag="sty")
                    nc.vector.tensor_mul(s_t[:], ps[:], rden[:])
                    sty.append(s_t)
                for j in range(KT):
                    ps = mmps.tile([P, 512], F32, tag="mmps")
                    for k in range(KT):
                        nc.tensor.matmul(
                            ps[:],
                            _r(ow_s[:, k * C + j * P: k * C + (j + 1) * P]),
                            _r(sty[k][:]),
                            start=(k == 0), stop=(k == KT - 1))
                    o_t = outp.tile([P, 512], F32, tag="outsb")
                    nc.scalar.activation(
                        o_t[:], ps[:], AF.Identity,
                        bias=pb_sb[:, 2 * KT + j: 2 * KT + j + 1])
                    nc.sync.dma_start(
                        out[j * P:(j + 1) * P, ch * 512:(ch + 1) * 512],
                        o_t[:])


def _kmajor(x, cols):
    """[KT*128, cols] -> [128, KT*cols] with column block k = rows k*128.."""
    return np.ascontiguousarray(
        np.asarray(x).reshape(KT, P, cols).transpose(1, 0, 2)
        .reshape(P, KT * cols), dtype=np.float32)


_NC_CACHE = None


def _get_nc():
    global _NC_CACHE
    if _NC_CACHE is None:
        _NC_CACHE = build_nc()
    return _NC_CACHE


def make_in_maps(content, style, f_w, f_b, g_w, g_b, h_w, h_b, out_w, out_b):
    b, Cc, H, W = content.shape
    hw = H * W
    cf = np.ascontiguousarray(content.reshape(b, Cc, hw), dtype=np.float32)
    sf = np.ascontiguousarray(style.reshape(b, Cc, hw), dtype=np.float32)
    pbias = np.concatenate([
        np.asarray(f_b, np.float32).reshape(KT, P).T,
        np.asarray(g_b, np.float32).reshape(KT, P).T,
        np.asarray(out_b, np.float32).reshape(KT, P).T], axis=1)
    rowconst = np.concatenate(
        [np.ones(P, np.float32), np.asarray(h_b, np.float32)]
    ).reshape(1, P + C)
    wT = {
        "fwT": _kmajor(np.asarray(f_w).T, C),
        "gwT": _kmajor(np.asarray(g_w).T, C),
        "hwT": _kmajor(np.asarray(h_w).T, C),
        "owT": _kmajor(np.asarray(out_w).T, C),
        "pbias": np.ascontiguousarray(pbias, dtype=np.float32),
        "rowconst": rowconst,
    }
    in_maps = []
    for core in range(8):
        bi, hi = core // 2, core % 2
        in_maps.append({
            "cA": _kmajor(cf[bi][:, hi * NLOC:(hi + 1) * NLOC], NLOC),
            "cB": _kmajor(cf[bi][:, (1 - hi) * NLOC:(2 - hi) * NLOC], NLOC),
            "style": _kmajor(sf[bi], hw),
            **wT,
        })
    return in_maps


def kernel(content, style, f_w, f_b, g_w, g_b, h_w, h_b, out_w, out_b):
    from concourse.bass_utils import run_bass_kernel_spmd

    global _LAST_IN_MAPS
    in_maps = make_in_maps(content, style, f_w, f_b, g_w, g_b, h_w, h_b,
                           out_w, out_b)
    _LAST_IN_MAPS = in_maps
    b, Cc, H, W = content.shape
    hw = H * W
    nc = _get_nc()
    res = run_bass_kernel_spmd(nc, in_maps, core_ids=list(range(8)))
    outf = np.empty((b, Cc, hw), dtype=np.float32)
    for core in range(8):
        bi, hi = core // 2, core % 2
        outf[bi][:, hi * NLOC:(hi + 1) * NLOC] = res.results[core]["out"]
    return outf.reshape(b, Cc, H, W)


# revision 36
# speedup vs baseline: 1.0162x; 1.0162x over previous
"""SANet-style attention (nn_Attention_1382979470038) on 8 TRN2 NeuronCores.

Sharding: 8 cores = 4 batches x 2 content-token halves (sequence parallel on
N, style tokens replicated within each pair).  No collectives: each core
computes output columns [C=512, N_loc=2048] of its batch independently.

Per-core math (M = 4096 style tokens, N_loc = 2048 content tokens):
  instance-norm folded into conv weights:  F = (f_w . rstd_c) @ x_half + f_b'
  G  = (g_w . rstd_s) @ style + g_b'      [C, M]   (c on partitions)
  Ht = style^T @ h_w^T + h_b              [M, C]   (m on partitions)
  St = G^T F                               [M, N]   (m on partitions)
  P  = exp(St)            (no max-subtraction: |S| <~ 30 is fp32-safe)
  den[n] = sum_m P[m,n]   (all-ones stationary matmul -> broadcast rows)
  out = out_w @ ((Ht^T P) . (1/den)) + out_b

float32r (FP22) matmuls for convs+scores; bf16 for the post-exp apply.
Inputs are host-rearranged to k-major [128, ...] layouts so each logical
load is a single DMA (DMA dispatch costs ~0.6us each on the sync queue).
"""

import sys

sys.path.insert(0, "/opt/trn_rl_repo")

import numpy as np

import concourse.bass as bass
import concourse.tile as tile
from concourse import mybir

P = 128
C = 512
HW = 4096
NLOC = 2048
EPS = 1e-5
KT = C // P        # 4 k-tiles of 128 channels
NCH = NLOC // 512  # 4 n-chunks of 512
MCH = HW // 512    # 8 m-chunks of 512
MT = HW // P       # 32 m-tiles of 128

F32 = mybir.dt.float32
F32R = mybir.dt.float32r
BF16 = mybir.dt.bfloat16

AF = mybir.ActivationFunctionType
ALU = mybir.AluOpType


def _r(ap):
    return ap.bitcast(F32R)


def build_nc(hoist=True):
    nc = bass.Bass()
    # k-major layouts: [128, KT * cols]; column block k holds rows k*128..
    cAp = nc.declare_dram_parameter("cA", [P, KT * NLOC], F32, isOutput=False)
    cBp = nc.declare_dram_parameter("cB", [P, KT * NLOC], F32, isOutput=False)
    stp = nc.declare_dram_parameter("style", [P, KT * HW], F32, isOutput=False)
    fwp_ = nc.declare_dram_parameter("fwT", [P, KT * C], F32, isOutput=False)
    gwp_ = nc.declare_dram_parameter("gwT", [P, KT * C], F32, isOutput=False)
    hwp_ = nc.declare_dram_parameter("hwT", [P, KT * C], F32, isOutput=False)
    owp_ = nc.declare_dram_parameter("owT", [P, KT * C], F32, isOutput=False)
    pbp = nc.declare_dram_parameter("pbias", [P, 3 * KT], F32, isOutput=False)
    rcp = nc.declare_dram_parameter("rowconst", [1, P + C], F32,
                                    isOutput=False)
    out = nc.declare_dram_parameter("out", [C, NLOC], F32, isOutput=True)

    with tile.TileContext(nc) as tc:
        _build(tc, cAp, cBp, stp, fwp_, gwp_, hwp_, owp_, pbp, rcp, out)
    if hoist:
        _hoist_excess_waits(nc)
    return nc


# Walrus caps sync-wait commands per instruction (Activation/TensorScalar fit
# only one).  Hoist excess waits onto injected same-engine NOPs placed just
# before the instruction: engines execute in order, so semantics match.
def _hoist_excess_waits(nc):
    ctr = [0]

    def mknop(engine, debug, waits, updates):
        ctr[0] += 1
        return mybir.InstNoOp(
            name=f"WH-{ctr[0]}", opcode="NoOp", engine=engine, debug=debug,
            ins=[], outs=[],
            sync_info=mybir.SyncInfo(on_wait=waits, on_update=updates),
        )

    for fn in nc.m.functions:
        for blk in fn.blocks:
            newl = []
            changed = False
            for inst in blk.instructions:
                si = getattr(inst, "sync_info", None)
                if si is not None and si.on_wait and len(si.on_wait) > 1:
                    waits = list(si.on_wait)
                    keep, hoist = waits[-1:], waits[:-1]
                    eng = getattr(inst, "engine", None)
                    for w in hoist:
                        newl.append(mknop(eng, inst.debug, [w], []))
                    inst.sync_info = mybir.SyncInfo(
                        on_wait=keep, on_update=list(si.on_update))
                    changed = True
                newl.append(inst)
            if changed:
                blk.instructions = newl


def _build(tc, cAp, cBp, stp, fwTp, gwTp, hwTp, owTp, pbp, rcp, out):
    nc = tc.nc
    from contextlib import ExitStack

    ctx = ExitStack()
    with ctx:
        # ---------- long-lived pools ----------
        small = ctx.enter_context(tc.tile_pool(name="small", bufs=96))
        cons = ctx.enter_context(tc.tile_pool(name="cons", bufs=1))
        fpool = ctx.enter_context(tc.tile_pool(name="fpool", bufs=4))
        owpool = ctx.enter_context(tc.tile_pool(name="owpool", bufs=1))
        # PSUM pools (8 banks: 4 + 2 + 1 + 1)
        stps = ctx.enter_context(tc.tile_pool(name="stps", bufs=4,
                                              space="PSUM"))
        mmps = ctx.enter_context(tc.tile_pool(name="mmps", bufs=2,
                                              space="PSUM"))
        denps = ctx.enter_context(tc.tile_pool(name="denps", bufs=1,
                                               space="PSUM"))
        biasps = ctx.enter_context(tc.tile_pool(name="biasps", bufs=1,
                                                space="PSUM"))

        # == phase 0: style streamed ONCE -> Ht conv + style stats ==========
        htpool = ctx.enter_context(tc.tile_pool(name="htpool", bufs=1))
        Ht_sb = htpool.tile([P, MT * C], BF16, tag="Ht")
        spieces = [[] for _ in range(KT)]

        def stat_piece(tslice, ncols, scratch_pool):
            """One [P, ncols] slice -> (sum, sumsq) [P,1] tiles."""
            s = small.tile([P, 1], F32, tag="stat")
            nc.vector.reduce_sum(s[:], tslice.bitcast(F32),
                                 axis=mybir.AxisListType.X)
            q = small.tile([P, 1], F32, tag="stat")
            scr = scratch_pool.tile([P, ncols], F32, tag="scr")
            nc.scalar.activation(scr[:], tslice.bitcast(F32), AF.Square,
                                 accum_out=q[:])
            return s, q

        def stat_final(pieces, total):
            ssum = pieces[0][0]
            for s, _ in pieces[1:]:
                t2 = small.tile([P, 1], F32, tag="stat")
                nc.vector.tensor_add(t2[:], ssum[:], s[:])
                ssum = t2
            qsum = pieces[0][1]
            for _, q in pieces[1:]:
                t2 = small.tile([P, 1], F32, tag="stat")
                nc.vector.tensor_add(t2[:], qsum[:], q[:])
                qsum = t2
            mean = small.tile([P, 1], F32R, tag="stat")
            nc.vector.tensor_scalar(mean[:], ssum[:], 1.0 / total, None,
                                    op0=ALU.mult)
            m2 = small.tile([P, 1], F32, tag="stat")
            nc.vector.tensor_mul(m2[:], mean[:].bitcast(F32),
                                 mean[:].bitcast(F32))
            v = small.tile([P, 1], F32, tag="stat")
            nc.vector.scalar_tensor_tensor(
                out=v[:], in0=m2[:], scalar=-float(total), in1=qsum[:],
                op0=ALU.mult, op1=ALU.add)
            varp = small.tile([P, 1], F32, tag="stat")
            nc.vector.tensor_scalar(varp[:], v[:], 1.0 / (total - 1), EPS,
                                    op0=ALU.mult, op1=ALU.add)
            std = small.tile([P, 1], F32, tag="stat")
            nc.scalar.activation(std[:], varp[:], AF.Sqrt)
            rstd = small.tile([P, 1], F32, tag="stat")
            nc.vector.reciprocal(rstd[:], std[:])
            return mean, rstd

        def bias_fixup(w_t, mean_tiles, pb_col0):
            """b'[o] = pbias[:, col] - sum_c w_scaled[c,o]*mean[c]."""
            outb = []
            for j in range(KT):
                ps = biasps.tile([P, 1], F32, tag="biasps")
                for k in range(KT):
                    nc.tensor.matmul(
                        ps[:],
                        w_t[:, k * C + j * P: k * C + (j + 1) * P]
                        .bitcast(F32),
                        mean_tiles[k][:].bitcast(F32),
                        start=(k == 0), stop=(k == KT - 1))
                bb = small.tile([P, 1], F32, tag="pb")
                nc.vector.tensor_sub(
                    bb[:], pb_sb[:, pb_col0 + j: pb_col0 + j + 1], ps[:])
                outb.append(bb)
            return outb

        def acc_piece(accs, k, piece):
            """Running (sum, sumsq) accumulation per k-tile."""
            if accs[k] is None:
                accs[k] = piece
            else:
                s0, q0 = accs[k]
                s1, q1 = piece
                s2 = small.tile([P, 1], F32, tag="stat")
                nc.vector.tensor_add(s2[:], s0[:], s1[:])
                q2 = small.tile([P, 1], F32, tag="stat")
                nc.vector.tensor_add(q2[:], q0[:], q1[:])
                accs[k] = (s2, q2)

        saccs = [None] * KT
        caccs = [None] * KT
        with tc.tile_pool(name="hwp", bufs=1) as hwp, \
             tc.tile_pool(name="sspool", bufs=2) as sspool, \
             tc.tile_pool(name="ssbfp", bufs=2) as ssbfp, \
             tc.tile_pool(name="fwpp", bufs=1) as fwpp, \
             tc.tile_pool(name="cbig", bufs=1) as cbig, \
             tc.tile_pool(name="cquart", bufs=2) as cquart, \
             tc.tile_pool(name="scratch0", bufs=2) as scratch0:
            hw_s = hwp.tile([P, KT * C], F32R, tag="hwT")
            nc.sync.dma_start(hw_s[:], hwTp[:, :].bitcast(F32R))
            hw_bf = hwp.tile([P, KT * C], BF16, tag="hwbf")
            nc.vector.tensor_copy(hw_bf[:], hw_s[:].bitcast(F32))
            rc_sb = cons.tile([1, P + C], F32R, tag="rc")
            nc.sync.dma_start(rc_sb[:], rcp[:, :].bitcast(F32R))
            ones_row = rc_sb[:, :P]
            hb_s = rc_sb[:, P:]
            pb_sb = cons.tile([P, 3 * KT], F32, tag="pb_sb")
            nc.sync.dma_start(pb_sb[:], pbp[:, :])
            ones_bf = cons.tile([P, P], BF16, tag="ones_bf")
            nc.vector.memset(ones_bf[:], 1.0)
            hb_bf = cons.tile([1, C], BF16, tag="hb_bf")
            nc.vector.tensor_copy(hb_bf[:], hb_s.bitcast(F32))

            stp3 = stp[:, :].rearrange("p (k m) -> p k m", k=KT)
            cA3 = cAp[:, :].rearrange("p (k m) -> p k m", k=KT)
            cB3 = cBp[:, :].rearrange("p (k m) -> p k m", k=KT)
            cA_s = cbig.tile([P, KT * NLOC], F32R, tag="cA")
            fw_raw = fwpp.tile([P, KT * C], F32, tag="fwr")
            fw_s = fwpp.tile([P, KT * C], F32R, tag="fws")
            cq = []
            for ch in range(MCH):
                sc = sspool.tile([P, KT * 512], F32R, tag="ss",
                                 name=f"ss{ch}")
                nc.sync.dma_start(
                    sc[:], stp3[:, :, ch * 512:(ch + 1) * 512].bitcast(F32R))
                scbf = ssbfp.tile([P, KT * 512], BF16, tag="ssbf",
                                  name=f"ssbf{ch}")
                nc.vector.tensor_copy(scbf[:], sc[:].bitcast(F32))
                for mi in range(4):
                    mt = ch * 4 + mi
                    ps = mmps.tile([P, 512], F32, tag="mmps")
                    for k in range(KT):
                        nc.tensor.matmul(
                            ps[:],
                            scbf[:, k * 512 + mi * P: k * 512 + (mi + 1) * P],
                            hw_bf[:, k * C:(k + 1) * C],
                            start=(k == 0), stop=False)
                    nc.tensor.matmul(ps[:], ones_bf[:1, :], hb_bf[:],
                                     start=False, stop=True)
                    nc.scalar.activation(
                        Ht_sb[:, mt * C:(mt + 1) * C], ps[:], AF.Copy)
                for k in range(KT):
                    acc_piece(saccs, k, stat_piece(
                        sc[:, k * 512:(k + 1) * 512], 512, scratch0))

            # content arrives after style on the DMA ring, in quarters;
            # stat pieces trail each quarter
            cA_v = cA_s[:].rearrange("p (k n) -> p k n", k=KT)
            for q in range(4):
                nc.sync.dma_start(
                    cA_v[:, :, q * 512:(q + 1) * 512],
                    cA3[:, :, q * 512:(q + 1) * 512].bitcast(F32R))
                qt = cquart.tile([P, KT * 512], F32, tag="cq", name=f"cq{q}")
                nc.sync.dma_start(qt[:], cB3[:, :, q * 512:(q + 1) * 512])
                cq.append(qt)
            nc.sync.dma_start(fw_raw[:], fwTp[:, :])
            for q in range(4):
                for k in range(KT):
                    acc_piece(caccs, k, stat_piece(
                        cA_s[:, k * NLOC + q * 512:
                             k * NLOC + (q + 1) * 512], 512, scratch0))
                    acc_piece(caccs, k, stat_piece(
                        cq[q][:, k * 512:(k + 1) * 512], 512, scratch0))
            smean, srstd = [], []
            for k in range(KT):
                mean, rstd = stat_final([saccs[k]], HW)
                smean.append(mean)
                srstd.append(rstd)

            # ============= content finalize + F conv =============
            with tc.tile_pool(name="scratch1", bufs=2) as scratch1:
                cmean, crstd = [], []
                for k in range(KT):
                    pends = [caccs[k]]
                    # remaining cA pieces (chunks NCH-? .. already covered 4,5,
                    # 6,7 -> (ch-4) in 0..3 == all 4 chunks of 512? NLOC=2048
                    # has 4 chunks; covered. )
                    mean, rstd = stat_final(pends, HW)
                    cmean.append(mean)
                    crstd.append(rstd)
                    nc.vector.tensor_scalar_mul(
                        fw_s[:, k * C:(k + 1) * C],
                        fw_raw[:, k * C:(k + 1) * C], crstd[k][:])
                fbp = bias_fixup(fw_s, cmean, 0)
                F_sb = [fpool.tile([P, NLOC], F32R, tag="F", name=f"F{k}")
                        for k in range(KT)]
                for ch in range(NCH):
                    for j in range(KT):
                        ps = stps.tile([P, 512], F32, tag="stps")
                        for k in range(KT):
                            nc.tensor.matmul(
                                ps[:],
                                _r(fw_s[:, k * C + j * P:
                                       k * C + (j + 1) * P]),
                                _r(cA_s[:, k * NLOC + ch * 512:
                                        k * NLOC + (ch + 1) * 512]),
                                start=(k == 0), stop=(k == KT - 1))
                        nc.scalar.activation(
                            F_sb[j][:, ch * 512:(ch + 1) * 512], ps[:],
                            AF.Identity, bias=fbp[j][:])

        # ========== phase 2: G conv (style re-streamed in chunk sets) ======
        gpool = ctx.enter_context(tc.tile_pool(name="gpool", bufs=4))
        G_sb = [gpool.tile([P, HW], F32R, tag="G", name=f"G{k}")
                for k in range(KT)]
        ghw_stack = ExitStack()
        ghwp = ghw_stack.enter_context(tc.tile_pool(name="ghwp", bufs=1))
        gw_raw = ghwp.tile([P, KT * C], F32, tag="gwr")
        nc.sync.dma_start(gw_raw[:], gwTp[:, :])
        gw_s = ghwp.tile([P, KT * C], F32R, tag="gws")
        for k in range(KT):
            nc.vector.tensor_scalar_mul(
                gw_s[:, k * C:(k + 1) * C],
                gw_raw[:, k * C:(k + 1) * C], srstd[k][:])
        gbp = bias_fixup(gw_s, smean, KT)
        with tc.tile_pool(name="schunk2", bufs=3) as schunk2:
            for ch in range(MCH):
                sc = schunk2.tile([P, KT * 512], F32R, tag="schunk",
                                  name=f"sg{ch}")
                nc.sync.dma_start(
                    sc[:],
                    stp[:, :].rearrange("p (k m) -> p k m", k=KT)
                    [:, :, ch * 512:(ch + 1) * 512].bitcast(F32R))
                for j in range(KT):
                    ps = stps.tile([P, 512], F32, tag="stps")
                    for k in range(KT):
                        nc.tensor.matmul(
                            ps[:],
                            _r(gw_s[:, k * C + j * P: k * C + (j + 1) * P]),
                            _r(sc[:, k * 512:(k + 1) * 512]),
                            start=(k == 0), stop=(k == KT - 1))
                    nc.scalar.activation(
                        G_sb[j][:, ch * 512:(ch + 1) * 512], ps[:],
                        AF.Identity, bias=gbp[j][:])
        ghw_stack.close()

        # ================= phase 3: attention =================
        ow_s = owpool.tile([P, KT * C], F32R, tag="owT")
        nc.sync.dma_start(ow_s[:], owTp[:, :].bitcast(F32R))
        with tc.tile_pool(name="expp", bufs=1) as expp, \
             tc.tile_pool(name="styp", bufs=4) as styp, \
             tc.tile_pool(name="rdenp", bufs=1) as rdenp, \
             tc.tile_pool(name="outp", bufs=2) as outp:
            for ch in range(NCH):
                exp_t = expp.tile([P, MT * 512], BF16, tag="exp")
                den = denps.tile([P, 512], F32, tag="den")
                for mt in range(MT):
                    ps = stps.tile([P, 512], F32, tag="stps")
                    for k in range(KT):
                        nc.tensor.matmul(
                            ps[:], _r(G_sb[k][:, mt * P:(mt + 1) * P]),
                            _r(F_sb[k][:, ch * 512:(ch + 1) * 512]),
                            start=(k == 0), stop=(k == KT - 1))
                    nc.scalar.activation(
                        exp_t[:, mt * 512:(mt + 1) * 512], ps[:], AF.Exp)
                    nc.tensor.matmul(
                        den[:], ones_bf[:],
                        exp_t[:, mt * 512:(mt + 1) * 512],
                        start=(mt == 0), stop=(mt == MT - 1))
                rden = rdenp.tile([P, 512], F32, tag="rden")
                nc.vector.reciprocal(rden[:], den[:])
                sty = []
                for j in range(KT):
                    ps = mmps.tile([P, 512], F32, tag="mmps")
                    for mt in range(MT):
                        nc.tensor.matmul(
                            ps[:],
                            Ht_sb[:, mt * C + j * P: mt * C + (j + 1) * P],
                            exp_t[:, mt * 512:(mt + 1) * 512],
                            start=(mt == 0), stop=(mt == MT - 1))
                    s_t = styp.tile([P, 512], F32R, tag="sty")
                    nc.vector.tensor_mul(s_t[:], ps[:], rden[:])
                    sty.append(s_t)
                for j in range(KT):
                    ps = mmps.tile([P, 512], F32, tag="mmps")
                    for k in range(KT):
                        nc.tensor.matmul(
                            ps[:],
                            _r(ow_s[:, k * C + j * P: k * C + (j + 1) * P]),
                            _r(sty[k][:]),
                            start=(k == 0), stop=(k == KT - 1))
                    o_t = outp.tile([P, 512], F32, tag="outsb")
                    nc.scalar.activation(
                        o_t[:], ps[:], AF.Identity,
                        bias=pb_sb[:, 2 * KT + j: 2 * KT + j + 1])
                    nc.sync.dma_start(
                        out[j * P:(j + 1) * P, ch * 512:(ch + 1) * 512],
                        o_t[:])


def _kmajor(x, cols):
    """[KT*128, cols] -> [128, KT*cols] with column block k = rows k*128.."""
    return np.ascontiguousarray(
        np.asarray(x).reshape(KT, P, cols).transpose(1, 0, 2)
        .reshape(P, KT * cols), dtype=np.float32)


_NC_CACHE = None


def _get_nc():
    global _NC_CACHE
    if _NC_CACHE is None:
        _NC_CACHE = build_nc()
    return _NC_CACHE


def make_in_maps(content, style, f_w, f_b, g_w, g_b, h_w, h_b, out_w, out_b):
    b, Cc, H, W = content.shape
    hw = H * W
    cf = np.ascontiguousarray(content.reshape(b, Cc, hw), dtype=np.float32)
    sf = np.ascontiguousarray(style.reshape(b, Cc, hw), dtype=np.float32)
    pbias = np.concatenate([
        np.asarray(f_b, np.float32).reshape(KT, P).T,
        np.asarray(g_b, np.float32).reshape(KT, P).T,
        np.asarray(out_b, np.float32).reshape(KT, P).T], axis=1)
    rowconst = np.concatenate(
        [np.ones(P, np.float32), np.asarray(h_b, np.float32)]
    ).reshape(1, P + C)
    wT = {
        "fwT": _kmajor(np.asarray(f_w).T, C),
        "gwT": _kmajor(np.asarray(g_w).T, C),
        "hwT": _kmajor(np.asarray(h_w).T, C),
        "owT": _kmajor(np.asarray(out_w).T, C),
        "pbias": np.ascontiguousarray(pbias, dtype=np.float32),
        "rowconst": rowconst,
    }
    in_maps = []
    for core in range(8):
        bi, hi = core // 2, core % 2
        in_maps.append({
            "cA": _kmajor(cf[bi][:, hi * NLOC:(hi + 1) * NLOC], NLOC),
            "cB": _kmajor(cf[bi][:, (1 - hi) * NLOC:(2 - hi) * NLOC], NLOC),
            "style": _kmajor(sf[bi], hw),
            **wT,
        })
    return in_maps


def kernel(content, style, f_w, f_b, g_w, g_b, h_w, h_b, out_w, out_b):
    from concourse.bass_utils import run_bass_kernel_spmd

    global _LAST_IN_MAPS
    in_maps = make_in_maps(content, style, f_w, f_b, g_w, g_b, h_w, h_b,
                           out_w, out_b)
    _LAST_IN_MAPS = in_maps
    b, Cc, H, W = content.shape
    hw = H * W
    nc = _get_nc()
    res = run_bass_kernel_spmd(nc, in_maps, core_ids=list(range(8)))
    outf = np.empty((b, Cc, hw), dtype=np.float32)
    for core in range(8):
        bi, hi = core // 2, core % 2
        outf[bi][:, hi * NLOC:(hi + 1) * NLOC] = res.results[core]["out"]
    return outf.reshape(b, Cc, H, W)
